# revision 12
# baseline (speedup 1.0000x reference)
"""Multihead attention (B=4, S=2048, E=1024, H=16, D=64) on 8 Trainium2 cores.

Sharding: core c = (batch b = c//2, head-half hh = c%2). Each core computes one
batch's attention for 8 heads (512 of the 1024 projection columns), producing a
partial output (row-split Wo); the host sums the two partials per batch.

Host-side prep: x is pre-transposed to xT [E, S] f16 (no on-chip transposes),
and the V-bias is folded into the output bias (bo' = bo + bv @ Wo), so the
kernel is pure matmul + softmax.

On-chip: qT/kT are [d, s]; scores [sk, sq]; softmax denominators ride along as
a ones column appended to V (M=65 matmul); exp needs no max subtraction since
scores ~ N(0,1). Normalization is deferred: fast-approx reciprocal of the
denominators, broadcast over d via K=1 PE matmuls.

Schedule: A_k, A_v upfront; then per 512-column chunk c: A_q(c) -> B(c)
(scores/exp/attnV, software-pipelined) -> A_q(c+1) -> norm(c) -> C(c) (out
projection + DMA). The interleave keeps the PE saturated so the HAM clock gate
stays at 2.4 GHz (the baseline spent 550us throttled at 1.2 GHz).

PSUM budget (8 banks): sc 2x2 + acc 2x1 + mix 2x1 = 8. The sc ring doubles as
the phase-A projection accumulator.
"""
import os
import sys

sys.path.insert(0, "/opt/trn_rl_repo")

import numpy as np

import concourse.bacc as bacc
import concourse.mybir as mybir
import concourse.tile as tile
from concourse.bass_utils import run_bass_kernel_spmd

E = 1024
H = 16
D = 64
B = 4
S = 2048
HH = E // 2          # projection cols per core
N_CORES = 8
P = 128
NCH = 4              # s-chunks of 512
CH = 512
f32 = mybir.dt.float32
f16 = mybir.dt.float16
i32 = mybir.dt.int32
AF = mybir.ActivationFunctionType

# Schraudolph fast-exp on the DVE: exp(x) ~= bitcast_f32(int32(A*x + Bc)).
# A folds the 0.125 score scale; C tuned for min RMS (~1.77%) on N(0,1) scores.
SCH_A = (1 << 23) * 1.4426950408889634 * 0.125
SCH_B = float((127 << 23) - 486500)
_DVE_EXP = os.environ.get("BASS_MHA_DVE_EXP", "1") == "1"

_cached = {}


def _build():
    mdt = f16
    nc = bacc.Bacc(None, target_bir_lowering=False)

    xqT = nc.declare_dram_parameter("xqT", [E, S], mdt, isOutput=False)
    xkT = nc.declare_dram_parameter("xkT", [E, S], mdt, isOutput=False)
    xvT = nc.declare_dram_parameter("xvT", [E, S], mdt, isOutput=False)
    wq = nc.declare_dram_parameter("wq", [P, 8, HH], mdt, isOutput=False)
    wk = nc.declare_dram_parameter("wk", [P, 8, HH], mdt, isOutput=False)
    wv = nc.declare_dram_parameter("wv", [P, 8, HH], mdt, isOutput=False)
    bq_col = nc.declare_dram_parameter("bq_col", [P, 4], f32, isOutput=False)
    bk_col = nc.declare_dram_parameter("bk_col", [P, 4], f32, isOutput=False)
    wo = nc.declare_dram_parameter("wo", [P, 4, E], mdt, isOutput=False)
    bo_col = nc.declare_dram_parameter("bo_col", [P, 8], f32, isOutput=False)
    yT = nc.declare_dram_parameter("yT", [E, S], f32, isOutput=True)

    from contextlib import ExitStack

    with tile.TileContext(nc) as tc, ExitStack() as stack:
        const = stack.enter_context(tc.tile_pool(name="const", bufs=1))
        qkv = stack.enter_context(tc.tile_pool(name="qkv", bufs=1))
        ps = stack.enter_context(tc.tile_pool(name="ps", bufs=2, space="PSUM"))
        xkvp = stack.enter_context(tc.tile_pool(name="xkv", bufs=1))
        xqp = stack.enter_context(tc.tile_pool(name="xq", bufs=2))
        exp_pool = stack.enter_context(tc.tile_pool(name="ex", bufs=6))
        oub = stack.enter_context(tc.tile_pool(name="oub", bufs=2))
        denp = stack.enter_context(tc.tile_pool(name="den", bufs=2))
        outb = stack.enter_context(tc.tile_pool(name="outb", bufs=3))

        onesf = const.tile([P, P], f32)
        nc.vector.memset(onesf[:], 1.0)
        pones_t = const.tile([P, P], mdt)      # rows 0/32/64/96: 1.0 (bcast lhsT)
        for r in (0, 32, 64, 96):
            nc.vector.tensor_copy(pones_t[r:r + 1, :], onesf[r:r + 1, :])
        vones = const.tile([P, 16, 8], mdt)    # ones column filler for vbuf
        nc.vector.memset(vones[:], 1.0)

        bqc = const.tile([P, 4], f32)
        bkc = const.tile([P, 4], f32)
        boc = const.tile([P, 8], f32)
        nc.sync.dma_start(out=bqc[:], in_=bq_col[:])
        nc.sync.dma_start(out=bkc[:], in_=bk_col[:])
        nc.sync.dma_start(out=boc[:], in_=bo_col[:])

        # persistent SBUF
        qT = qkv.tile([P, 4, S], mdt)            # [d within pair, pair, sq]
        kT = qkv.tile([P, 4, S], mdt)
        vbuf = qkv.tile([P, 16, 8, D + 1], mdt)  # [sv, s-tile, head, d|1]
        nc.vector.tensor_copy(vbuf[:, :, :, D], vones[:])

        # weights (prefetch everything at the start; gpsimd queue)
        wq_t = qkv.tile([P, 8, HH], mdt)
        wk_t = qkv.tile([P, 8, HH], mdt)
        wv_t = qkv.tile([P, 8, HH], mdt)
        wo_t = qkv.tile([P, 4, E], mdt)
        nc.gpsimd.dma_start(out=wk_t[:], in_=wk[:])
        nc.gpsimd.dma_start(out=wv_t[:], in_=wv[:])
        nc.gpsimd.dma_start(out=wq_t[:], in_=wq[:])
        nc.gpsimd.dma_start(out=wo_t[:], in_=wo[:])

        # ---------------- Phase A: k then v projections ----------------
        # xk arrives chunk-major so A_k(c=0) can start after ~1MB instead of 4MB
        xk_sb = xkvp.tile([P, 8, S], mdt, tag="x", name="xk_sb")
        for c in range(NCH):
            for el in range(8):
                nc.gpsimd.dma_start(out=xk_sb[:, el, c * CH:(c + 1) * CH],
                                    in_=xkT[el * P:(el + 1) * P, c * CH:(c + 1) * CH])
        for c in range(NCH):
            cs = slice(c * CH, (c + 1) * CH)
            for u in range(4):
                pp = ps.tile([P, CH], f32, tag=("sc" if u % 2 == 0 else "mix"), name="ppk")
                for el in range(8):
                    nc.tensor.matmul(pp[:], lhsT=wk_t[:, el, u * P:(u + 1) * P],
                                     rhs=xk_sb[:, el, cs],
                                     start=(el == 0), stop=(el == 7))
                nc.vector.tensor_scalar_add(kT[:, u, cs], pp[:], bkc[:, u:u + 1])

        xv_sb = xkvp.tile([P, 8, S], mdt, tag="x", name="xv_sb")
        for c in range(NCH):
            for el in range(8):
                nc.gpsimd.dma_start(out=xv_sb[:, el, c * CH:(c + 1) * CH],
                                    in_=xvT[el * P:(el + 1) * P, c * CH:(c + 1) * CH])
        for st in range(16):
            pp = ps.tile([P, 8, D], f32, tag=("sc" if st % 2 == 0 else "mix"), name="ppv")
            for el in range(8):
                nc.tensor.matmul(pp[:], lhsT=xv_sb[:, el, st * P:(st + 1) * P],
                                 rhs=wv_t[:, el, :],
                                 start=(el == 0), stop=(el == 7))
            nc.vector.tensor_copy(vbuf[:, st, :, 0:D], pp[:])

        # xq chunk prefetch (sync queue; overlaps with A_k/A_v compute)
        def xq_load(c):
            cs = slice(c * CH, (c + 1) * CH)
            xq_sb = xqp.tile([P, 8, CH], mdt, tag="xq", name=f"xq{c}")
            for el in range(8):
                nc.sync.dma_start(out=xq_sb[:, el, :], in_=xqT[el * P:(el + 1) * P, cs])
            return xq_sb

        def a_q(c, xq_sb):
            cs = slice(c * CH, (c + 1) * CH)
            for u in range(4):
                pp = ps.tile([P, CH], f32, tag=("sc" if u % 2 == 0 else "mix"), name="ppq")
                for el in range(8):
                    nc.tensor.matmul(pp[:], lhsT=wq_t[:, el, u * P:(u + 1) * P],
                                     rhs=xq_sb[:, el, :],
                                     start=(el == 0), stop=(el == 7))
                nc.vector.tensor_scalar_add(qT[:, u, cs], pp[:], bqc[:, u:u + 1])

        xq_tiles = {c: xq_load(c) for c in (0, 1)}
        a_q(0, xq_tiles[0])

        # ---------------- chunk loop: B (attention) + norm + C ----------------
        for c in range(NCH):
            cs = slice(c * CH, (c + 1) * CH)
            ou = oub.tile([P, 4, CH], mdt, tag="ou", name=f"ou{c}")
            den = denp.tile([P, 2, CH], f32, tag="den", name=f"den{c}")
            den_r = denp.tile([P, 2, CH], f32, tag="denr", bufs=1, name="den_r")
            den_h = denp.tile([P, 2, CH], mdt, tag="denh", name="den_h")

            def norm_pr(pr):
                # 1/den (approx; NB: custom-DVE ops only work at partition base
                # 0, so recip the whole tile — garbage rows are unused),
                # broadcast over d via K=1 matmul, ou *= 1/den
                sl2 = pr // 2
                with nc.allow_low_precision(reason="softmax scale factors"):
                    nc.vector.reciprocal_approx_fast(den_r[:], den[:])
                    for half in range(2):
                        r = 32 * ((pr % 2) * 2 + half)
                        hs = slice(64 * half, 64 * half + 64)
                        nc.vector.tensor_copy(den_h[r:r + 1, sl2, :],
                                              den_r[r:r + 1, sl2, :])
                        psb = ps.tile([64, CH], f32, tag="mix", name="psb")
                        nc.tensor.matmul(psb[:], lhsT=pones_t[r:r + 1, 0:64],
                                         rhs=den_h[r:r + 1, sl2, :],
                                         start=True, stop=True,
                                         tile_position=(r, 0))
                        nc.vector.tensor_mul(ou[hs, pr, :], ou[hs, pr, :], psb[:])

            for pr in range(4):
                hA, hB = 2 * pr, 2 * pr + 1
                psoA = ps.tile([D + 1, CH], f32, tag="acc", name="psoA")
                psoB = ps.tile([D + 1, CH], f32, tag="acc", name="psoB")
                exbuf = {}

                def emit_scores(i):
                    tiles = []
                    for half in (0, 1):
                        pbs = slice(64 * half, 64 * half + 64)
                        psc = ps.tile([P, 2, CH], f32, tag="sc", name=f"psc{half}")
                        for j in (0, 1):
                            st = 2 * i + j
                            nc.tensor.matmul(psc[:, j, :],
                                             lhsT=kT[pbs, pr, st * P:(st + 1) * P],
                                             rhs=qT[pbs, pr, cs],
                                             start=True, stop=True)
                        ex = exp_pool.tile([P, 2, CH], mdt, tag="ex", name=f"ex{half}")
                        if _DVE_EXP and (2 * i + half + pr) % 4 == 3:
                            # Schraudolph fast-exp on the DVE (~1.8% rms noise on
                            # 25% of tiles) to unload the Scalar engine
                            t32 = exp_pool.tile([P, 2, CH], i32, tag="i32", bufs=2,
                                                name="t32")
                            nc.vector.tensor_scalar(
                                out=t32[:], in0=psc[:], scalar1=SCH_A, scalar2=SCH_B,
                                op0=mybir.AluOpType.mult, op1=mybir.AluOpType.add)
                            nc.vector.tensor_copy(ex[:], t32[:].bitcast(f32))
                        else:
                            nc.scalar.activation(ex[:], psc[:], AF.Exp, scale=0.125)
                        tiles.append(ex)
                    exbuf[i] = tiles

                def emit_attnv(i):
                    exA, exB = exbuf.pop(i)
                    for pso, ex, hh_ in ((psoA, exA, hA), (psoB, exB, hB)):
                        for j in (0, 1):
                            st = 2 * i + j
                            nc.tensor.matmul(pso[:], lhsT=vbuf[:, st, hh_, :],
                                             rhs=ex[:, j, :],
                                             start=(st == 0), stop=(st == 15),
                                             skip_group_check=True)

                emit_scores(0)
                for i in range(8):
                    if i < 7:
                        emit_scores(i + 1)
                    emit_attnv(i)

                # stash unnormalized output + denominators
                nc.vector.tensor_copy(ou[0:64, pr, :], psoA[0:64, :])
                nc.vector.tensor_copy(ou[64:128, pr, :], psoB[0:64, :])
                rA = 32 * ((pr % 2) * 2 + 0)
                rB = 32 * ((pr % 2) * 2 + 1)
                sl2 = pr // 2
                nc.vector.tensor_copy(den[rA:rA + 1, sl2, :], psoA[64:65, :])
                nc.vector.tensor_copy(den[rB:rB + 1, sl2, :], psoB[64:65, :])
                if c == NCH - 1:
                    # last chunk: normalize per-pr so C isn't serialized behind
                    # a whole-chunk DVE chain at the very end
                    norm_pr(pr)

            # PE filler while the DVE does the reciprocal chain
            if c + 1 < NCH:
                a_q(c + 1, xq_tiles.pop(c + 1))
            if c + 2 < NCH:
                xq_tiles[c + 2] = xq_load(c + 2)

            if c != NCH - 1:
                # single recip + cast for the whole chunk
                with nc.allow_low_precision(reason="softmax scale factors"):
                    nc.vector.reciprocal_approx_fast(den_r[:], den[:])
                    nc.vector.tensor_copy(den_h[:], den_r[:])
                    for pr in range(4):
                        sl2 = pr // 2
                        for half in range(2):
                            r = 32 * ((pr % 2) * 2 + half)
                            hs = slice(64 * half, 64 * half + 64)
                            psb = ps.tile([64, CH], f32, tag="mix", name="psb")
                            nc.tensor.matmul(psb[:], lhsT=pones_t[r:r + 1, 0:64],
                                             rhs=den_h[r:r + 1, sl2, :],
                                             start=True, stop=True,
                                             tile_position=(r, 0))
                            nc.vector.tensor_mul(ou[hs, pr, :], ou[hs, pr, :],
                                                 psb[:])

            # ---------------- C: output projection for this chunk ----------------
            for et in range(8):
                po = ps.tile([P, CH], f32, tag="mix", name="po")
                for t in range(4):
                    nc.tensor.matmul(po[:], lhsT=wo_t[:, t, et * P:(et + 1) * P],
                                     rhs=ou[:, t, :],
                                     start=(t == 0), stop=(t == 3))
                out_t = outb.tile([P, CH], f32, tag="out", name="out_t")
                nc.vector.tensor_scalar_add(out_t[:], po[:], boc[:, et:et + 1])
                nc.sync.dma_start(out=yT[et * P:(et + 1) * P, cs], in_=out_t[:])

    nc.finalize()
    return nc


def _get_nc():
    if "nc" not in _cached:
        _cached["nc"] = _build()
    return _cached["nc"]


def _in_maps(query, key, value, Wq, bq, Wk, bk, Wv, bv, Wo, bo):
    query = np.asarray(query, np.float32)
    key = np.asarray(key, np.float32)
    value = np.asarray(value, np.float32)
    Wo = np.asarray(Wo, np.float32)
    bv = np.asarray(bv, np.float32)
    bo = np.asarray(bo, np.float32)

    xT = {}
    for b in range(B):
        xT[b] = (np.ascontiguousarray(query[b].T).astype(np.float16),
                 np.ascontiguousarray(key[b].T).astype(np.float16),
                 np.ascontiguousarray(value[b].T).astype(np.float16))

    maps = []
    for c in range(N_CORES):
        b, hh = divmod(c, 2)
        sl = slice(hh * HH, (hh + 1) * HH)

        def wcols(W):
            Ws = np.asarray(W, np.float32)[:, sl]
            return np.ascontiguousarray(
                Ws.reshape(8, P, HH).transpose(1, 0, 2)).astype(np.float16)

        wo_s = Wo[sl, :]                                              # [512, E]
        wo_r = np.ascontiguousarray(
            wo_s.reshape(4, P, E).transpose(1, 0, 2)).astype(np.float16)
        # fold the V-bias through the output projection: bo' = bo + bv @ Wo
        bo_eff = bv[sl] @ wo_s + (bo if hh == 0 else 0.0)
        bo_c = np.ascontiguousarray(bo_eff.reshape(8, P).T.astype(np.float32))
        xq_b, xk_b, xv_b = xT[b]
        maps.append({
            "xqT": xq_b,
            "xkT": xk_b,
            "xvT": xv_b,
            "wq": wcols(Wq),
            "wk": wcols(Wk),
            "wv": wcols(Wv),
            "bq_col": np.ascontiguousarray(np.asarray(bq, np.float32)[sl].reshape(4, P).T),
            "bk_col": np.ascontiguousarray(np.asarray(bk, np.float32)[sl].reshape(4, P).T),
            "wo": wo_r,
            "bo_col": bo_c,
        })
    return maps


def _assemble(results):
    outs = [results[c]["yT"] for c in range(N_CORES)]
    return np.stack([(outs[2 * b] + outs[2 * b + 1]).T for b in range(B)]).astype(np.float32)


def kernel(**inputs):
    nc = _get_nc()
    maps = _in_maps(**inputs)
    r = run_bass_kernel_spmd(nc, maps, list(range(N_CORES)))
    return _assemble(r.results)


def _ensure_ntff_hook():
    """Register the axon NTFF profiling hook (missing antenv.axon_hooks shim)."""
    import contextlib
    import ctypes
    import types

    try:
        from antenv.axon_hooks import get_axon_ntff_profile_hook
        if get_axon_ntff_profile_hook() is not None:
            return
    except ImportError:
        pass

    import antenv

    holder = {}
    mod = types.ModuleType("antenv.axon_hooks")
    mod.set_axon_ntff_profile_hook = lambda h: holder.__setitem__("h", h)
    mod.get_axon_ntff_profile_hook = lambda: holder.get("h")
    sys.modules["antenv.axon_hooks"] = mod
    antenv.axon_hooks = mod

    so_path = "/opt/axon/libaxon_pjrt.so"
    lib = ctypes.CDLL(so_path)
    if not hasattr(lib, "axon_start_nrt_profile"):
        return
    lib.axon_start_nrt_profile.argtypes = [ctypes.POINTER(ctypes.c_int64), ctypes.c_size_t]
    lib.axon_start_nrt_profile.restype = ctypes.c_int64
    lib.axon_stop_nrt_profile.argtypes = [ctypes.c_char_p]
    lib.axon_stop_nrt_profile.restype = ctypes.c_int64

    @contextlib.contextmanager
    def _hook(output_dir, device_ids):
        import jax

        jax.devices()
        if device_ids:
            ids = (ctypes.c_int64 * len(device_ids))(*device_ids)
            rc = lib.axon_start_nrt_profile(ids, len(device_ids))
        else:
            rc = lib.axon_start_nrt_profile(None, 0)
        if rc != 0:
            raise RuntimeError(f"axon_start_nrt_profile rc={rc}")
        try:
            yield
        finally:
            n = lib.axon_stop_nrt_profile(str(output_dir).encode())
            if n < 0:
                raise RuntimeError(f"axon_stop_nrt_profile rc={n}")

    mod.set_axon_ntff_profile_hook(_hook)


def kernel_traced(tmpdir=None, **inputs):
    """Like kernel() but with NTFF tracing; returns (output, exec_time_ns)."""
    _ensure_ntff_hook()
    import concourse.bass_utils as bu
    bu.upload_artifacts = lambda d: d  # no artifact bucket in this container
    nc = _get_nc()
    maps = _in_maps(**inputs)
    r = run_bass_kernel_spmd(nc, maps, list(range(N_CORES)), trace=True, tmpdir=tmpdir)
    return _assemble(r.results), r.exec_time_ns


# revision 15
# speedup vs baseline: 1.3227x; 1.3227x over previous
"""Multihead attention (B=4, S=2048, E=1024, H=16, D=64) on 8 Trainium2 cores.

Sharding: core c = (batch b = c//2, head-half hh = c%2). Each core computes one
batch's attention for 8 heads (512 of the 1024 projection columns), producing a
partial output (row-split Wo); the host sums the two partials per batch.

Host-side prep: x is pre-transposed to xT [E, S] f16 (no on-chip transposes),
and the V-bias is folded into the output bias (bo' = bo + bv @ Wo), so the
kernel is pure matmul + softmax.

On-chip: qT/kT are [d, s]; scores [sk, sq]; softmax denominators ride along as
a ones column appended to V (M=65 matmul); exp needs no max subtraction since
scores ~ N(0,1). Normalization is deferred: fast-approx reciprocal of the
denominators, broadcast over d via K=1 PE matmuls.

Schedule: A_k, A_v upfront; then per 512-column chunk c: A_q(c) -> B(c)
(scores/exp/attnV, software-pipelined) -> A_q(c+1) -> norm(c) -> C(c) (out
projection + DMA). The interleave keeps the PE saturated so the HAM clock gate
stays at 2.4 GHz (the baseline spent 550us throttled at 1.2 GHz).

PSUM budget (8 banks): sc 2x2 + acc 2x1 + mix 2x1 = 8. The sc ring doubles as
the phase-A projection accumulator.
"""
import os
import sys

sys.path.insert(0, "/opt/trn_rl_repo")

import numpy as np

import concourse.bacc as bacc
import concourse.mybir as mybir
import concourse.tile as tile
from concourse.bass_utils import run_bass_kernel_spmd

E = 1024
H = 16
D = 64
B = 4
S = 2048
HH = E // 2          # projection cols per core
N_CORES = 8
P = 128
NCH = 4              # s-chunks of 512
CH = 512
f32 = mybir.dt.float32
f16 = mybir.dt.float16
i32 = mybir.dt.int32
AF = mybir.ActivationFunctionType

# Schraudolph fast-exp on the DVE: exp(x) ~= bitcast_f32(int32(A*x + Bc)).
# A folds the 0.125 score scale; C tuned for min RMS (~1.77%) on N(0,1) scores.
SCH_A = (1 << 23) * 1.4426950408889634 * 0.125
SCH_B = float((127 << 23) - 486500)
_DVE_EXP = os.environ.get("BASS_MHA_DVE_EXP", "0") == "1"

_cached = {}


def _build():
    mdt = f16
    nc = bacc.Bacc(None, target_bir_lowering=False)

    xqT = nc.declare_dram_parameter("xqT", [E, S], mdt, isOutput=False)
    xkT = nc.declare_dram_parameter("xkT", [E, S], mdt, isOutput=False)
    xvT = nc.declare_dram_parameter("xvT", [E, S], mdt, isOutput=False)
    wq = nc.declare_dram_parameter("wq", [P, 8, HH], mdt, isOutput=False)
    wk = nc.declare_dram_parameter("wk", [P, 8, HH], mdt, isOutput=False)
    wv = nc.declare_dram_parameter("wv", [P, 8, HH], mdt, isOutput=False)
    bq_col = nc.declare_dram_parameter("bq_col", [P, 4], f32, isOutput=False)
    bk_col = nc.declare_dram_parameter("bk_col", [P, 4], f32, isOutput=False)
    wo = nc.declare_dram_parameter("wo", [P, 4, E], mdt, isOutput=False)
    bo_col = nc.declare_dram_parameter("bo_col", [P, 8], f32, isOutput=False)
    yT = nc.declare_dram_parameter("yT", [E, S], f32, isOutput=True)

    from contextlib import ExitStack

    with tile.TileContext(nc) as tc, ExitStack() as stack:
        const = stack.enter_context(tc.tile_pool(name="const", bufs=1))
        qkv = stack.enter_context(tc.tile_pool(name="qkv", bufs=1))
        ps = stack.enter_context(tc.tile_pool(name="ps", bufs=2, space="PSUM"))
        xkvp = stack.enter_context(tc.tile_pool(name="xkv", bufs=1))
        xqp = stack.enter_context(tc.tile_pool(name="xq", bufs=2))
        exp_pool = stack.enter_context(tc.tile_pool(name="ex", bufs=6))
        oub = stack.enter_context(tc.tile_pool(name="oub", bufs=2))
        denp = stack.enter_context(tc.tile_pool(name="den", bufs=2))
        outb = stack.enter_context(tc.tile_pool(name="outb", bufs=3))

        onesf = const.tile([P, P], f32)
        nc.vector.memset(onesf[:], 1.0)
        pones_t = const.tile([P, P], mdt)      # rows 0/32/64/96: 1.0 (bcast lhsT)
        for r in (0, 32, 64, 96):
            nc.vector.tensor_copy(pones_t[r:r + 1, :], onesf[r:r + 1, :])
        vones = const.tile([P, 16, 8], mdt)    # ones column filler for vbuf
        nc.vector.memset(vones[:], 1.0)

        bqc = const.tile([P, 4], f32)
        bkc = const.tile([P, 4], f32)
        boc = const.tile([P, 8], f32)
        nc.sync.dma_start(out=bqc[:], in_=bq_col[:])
        nc.sync.dma_start(out=bkc[:], in_=bk_col[:])
        nc.sync.dma_start(out=boc[:], in_=bo_col[:])

        # persistent SBUF
        qT = qkv.tile([P, 4, S], mdt)            # [d within pair, pair, sq]
        kT = qkv.tile([P, 4, S], mdt)
        vbuf = qkv.tile([P, 16, 8, D + 1], mdt)  # [sv, s-tile, head, d|1]
        nc.vector.tensor_copy(vbuf[:, :, :, D], vones[:])

        # weights: wk first (A_k needs it immediately), then x; wq/wo ride the
        # sync queue behind the small biases
        wq_t = qkv.tile([P, 8, HH], mdt)
        wk_t = qkv.tile([P, 8, HH], mdt)
        wv_t = qkv.tile([P, 8, HH], mdt)
        wo_t = qkv.tile([P, 4, E], mdt)
        nc.gpsimd.dma_start(out=wk_t[:], in_=wk[:])
        nc.sync.dma_start(out=wq_t[:], in_=wq[:])

        # ---------------- Phase A: k then v projections ----------------
        # xk arrives chunk-major so A_k(c=0) can start after ~1MB instead of 4MB
        xk_sb = xkvp.tile([P, 8, S], mdt, tag="x", name="xk_sb")
        for c in range(NCH):
            for el in range(8):
                nc.gpsimd.dma_start(out=xk_sb[:, el, c * CH:(c + 1) * CH],
                                    in_=xkT[el * P:(el + 1) * P, c * CH:(c + 1) * CH])
        nc.gpsimd.dma_start(out=wv_t[:], in_=wv[:])
        for c in range(NCH):
            cs = slice(c * CH, (c + 1) * CH)
            for u in range(4):
                pp = ps.tile([P, CH], f32, tag=("sc" if u % 2 == 0 else "mix"), name="ppk")
                for el in range(8):
                    nc.tensor.matmul(pp[:], lhsT=wk_t[:, el, u * P:(u + 1) * P],
                                     rhs=xk_sb[:, el, cs],
                                     start=(el == 0), stop=(el == 7))
                nc.vector.tensor_scalar_add(kT[:, u, cs], pp[:], bkc[:, u:u + 1])

        xv_sb = xkvp.tile([P, 8, S], mdt, tag="x", name="xv_sb")
        for c in range(NCH):
            for el in range(8):
                nc.gpsimd.dma_start(out=xv_sb[:, el, c * CH:(c + 1) * CH],
                                    in_=xvT[el * P:(el + 1) * P, c * CH:(c + 1) * CH])
        nc.sync.dma_start(out=wo_t[:], in_=wo[:])
        for st in range(16):
            pp = ps.tile([P, 8, D], f32, tag=("sc" if st % 2 == 0 else "mix"), name="ppv")
            for el in range(8):
                nc.tensor.matmul(pp[:], lhsT=xv_sb[:, el, st * P:(st + 1) * P],
                                 rhs=wv_t[:, el, :],
                                 start=(el == 0), stop=(el == 7))
            nc.vector.tensor_copy(vbuf[:, st, :, 0:D], pp[:])

        # xq chunk prefetch (sync queue; overlaps with A_k/A_v compute)
        def xq_load(c):
            cs = slice(c * CH, (c + 1) * CH)
            xq_sb = xqp.tile([P, 8, CH], mdt, tag="xq", name=f"xq{c}")
            for el in range(8):
                nc.sync.dma_start(out=xq_sb[:, el, :], in_=xqT[el * P:(el + 1) * P, cs])
            return xq_sb

        def a_q_group(c, xq_sb, u):
            cs = slice(c * CH, (c + 1) * CH)
            pp = ps.tile([P, CH], f32, tag="mix", name="ppq")
            for el in range(8):
                nc.tensor.matmul(pp[:], lhsT=wq_t[:, el, u * P:(u + 1) * P],
                                 rhs=xq_sb[:, el, :],
                                 start=(el == 0), stop=(el == 7))
            nc.vector.tensor_scalar_add(qT[:, u, cs], pp[:], bqc[:, u:u + 1])

        def norm_all(ou_c, den_c):
            # 1/den (approx; custom-DVE ops only work at partition base 0, so
            # recip the whole tile — garbage rows are unused), broadcast over d
            # via K=1 matmuls, ou *= 1/den
            den_r = denp.tile([P, 2, CH], f32, tag="denr", bufs=1, name="den_r")
            den_h = denp.tile([P, 2, CH], mdt, tag="denh", name="den_h")
            with nc.allow_low_precision(reason="softmax scale factors"):
                nc.vector.reciprocal_approx_fast(den_r[:], den_c[:])
                nc.vector.tensor_copy(den_h[:], den_r[:])
                for pr in range(4):
                    sl2 = pr // 2
                    for half in range(2):
                        r = 32 * ((pr % 2) * 2 + half)
                        hs = slice(64 * half, 64 * half + 64)
                        psb = ps.tile([64, CH], f32, tag="mix", name="psb")
                        nc.tensor.matmul(psb[:], lhsT=pones_t[r:r + 1, 0:64],
                                         rhs=den_h[r:r + 1, sl2, :],
                                         start=True, stop=True,
                                         tile_position=(r, 0))
                        nc.vector.tensor_mul(ou_c[hs, pr, :], ou_c[hs, pr, :],
                                             psb[:])

        def c_group(c, ou_c, et):
            cs = slice(c * CH, (c + 1) * CH)
            po = ps.tile([P, CH], f32, tag="mix", name="po")
            for t in range(4):
                nc.tensor.matmul(po[:], lhsT=wo_t[:, t, et * P:(et + 1) * P],
                                 rhs=ou_c[:, t, :],
                                 start=(t == 0), stop=(t == 3))
            out_t = outb.tile([P, CH], f32, tag="out", name="out_t")
            nc.vector.tensor_scalar_add(out_t[:], po[:], boc[:, et:et + 1])
            nc.sync.dma_start(out=yT[et * P:(et + 1) * P, cs], in_=out_t[:])

        xq_tiles = {c: xq_load(c) for c in (0, 1)}
        for u in range(4):
            a_q_group(0, xq_tiles[0], u)

        # ---------------- chunk loop ----------------
        # B(c) is Scalar(exp)-paced with ~25% PE slack; the previous chunk's
        # norm + output projection and the next chunk's q-projection are
        # emitted as filler groups inside B(c)'s matmul stream so the PE (and
        # the HAM clock) never go idle.  slot = pr*8 + i runs 0..31.
        prev = None  # (c, ou, den) of the previous chunk
        for c in range(NCH):
            cs = slice(c * CH, (c + 1) * CH)
            ou = oub.tile([P, 4, CH], mdt, tag="ou", name=f"ou{c}")
            den = denp.tile([P, 2, CH], f32, tag="den", name=f"den{c}")
            if c + 2 < NCH:
                xq_tiles[c + 2] = xq_load(c + 2)

            fillers = {}
            if prev is not None:
                pc, pou, pden = prev
                fillers[3] = lambda: norm_all(pou, pden)
                for et in range(8):
                    fillers[6 + 2 * et] = (
                        lambda et=et, pc=pc, pou=pou: c_group(pc, pou, et))
            if c + 1 < NCH:
                xq_next = xq_tiles.pop(c + 1)
                for u in range(4):
                    fillers[23 + 2 * u] = (
                        lambda u=u, xq=xq_next: a_q_group(c + 1, xq, u))

            for pr in range(4):
                hA, hB = 2 * pr, 2 * pr + 1
                psoA = ps.tile([D + 1, CH], f32, tag="acc", name="psoA")
                psoB = ps.tile([D + 1, CH], f32, tag="acc", name="psoB")
                exbuf = {}

                def emit_scores(i):
                    tiles = []
                    for half in (0, 1):
                        pbs = slice(64 * half, 64 * half + 64)
                        psc = ps.tile([P, 2, CH], f32, tag="sc", name=f"psc{half}")
                        for j in (0, 1):
                            st = 2 * i + j
                            nc.tensor.matmul(psc[:, j, :],
                                             lhsT=kT[pbs, pr, st * P:(st + 1) * P],
                                             rhs=qT[pbs, pr, cs],
                                             start=True, stop=True)
                        ex = exp_pool.tile([P, 2, CH], mdt, tag="ex", name=f"ex{half}")
                        if _DVE_EXP and (2 * i + half + pr) % 4 == 3:
                            # Schraudolph fast-exp on the DVE (~1.8% rms noise)
                            t32 = exp_pool.tile([P, 2, CH], i32, tag="i32", bufs=2,
                                                name="t32")
                            nc.vector.tensor_scalar(
                                out=t32[:], in0=psc[:], scalar1=SCH_A, scalar2=SCH_B,
                                op0=mybir.AluOpType.mult, op1=mybir.AluOpType.add)
                            nc.vector.tensor_copy(ex[:], t32[:].bitcast(f32))
                        else:
                            nc.scalar.activation(ex[:], psc[:], AF.Exp, scale=0.125)
                        tiles.append(ex)
                    exbuf[i] = tiles

                def emit_attnv(i):
                    exA, exB = exbuf.pop(i)
                    for pso, ex, hh_ in ((psoA, exA, hA), (psoB, exB, hB)):
                        for j in (0, 1):
                            st = 2 * i + j
                            nc.tensor.matmul(pso[:], lhsT=vbuf[:, st, hh_, :],
                                             rhs=ex[:, j, :],
                                             start=(st == 0), stop=(st == 15),
                                             skip_group_check=True)

                emit_scores(0)
                for i in range(8):
                    if i < 7:
                        emit_scores(i + 1)
                    emit_attnv(i)
                    f = fillers.pop(pr * 8 + i, None)
                    if f is not None:
                        f()

                # stash unnormalized output + denominators
                nc.vector.tensor_copy(ou[0:64, pr, :], psoA[0:64, :])
                nc.vector.tensor_copy(ou[64:128, pr, :], psoB[0:64, :])
                rA = 32 * ((pr % 2) * 2 + 0)
                rB = 32 * ((pr % 2) * 2 + 1)
                sl2 = pr // 2
                nc.vector.tensor_copy(den[rA:rA + 1, sl2, :], psoA[64:65, :])
                nc.vector.tensor_copy(den[rB:rB + 1, sl2, :], psoB[64:65, :])

            prev = (c, ou, den)

        # tail: last chunk's norm + output projection
        pc, pou, pden = prev
        norm_all(pou, pden)
        for et in range(8):
            c_group(pc, pou, et)

    nc.finalize()
    return nc


def _get_nc():
    if "nc" not in _cached:
        _cached["nc"] = _build()
    return _cached["nc"]


def _in_maps(query, key, value, Wq, bq, Wk, bk, Wv, bv, Wo, bo):
    query = np.asarray(query, np.float32)
    key = np.asarray(key, np.float32)
    value = np.asarray(value, np.float32)
    Wo = np.asarray(Wo, np.float32)
    bv = np.asarray(bv, np.float32)
    bo = np.asarray(bo, np.float32)

    xT = {}
    for b in range(B):
        xT[b] = (np.ascontiguousarray(query[b].T).astype(np.float16),
                 np.ascontiguousarray(key[b].T).astype(np.float16),
                 np.ascontiguousarray(value[b].T).astype(np.float16))

    maps = []
    for c in range(N_CORES):
        b, hh = divmod(c, 2)
        sl = slice(hh * HH, (hh + 1) * HH)

        def wcols(W):
            Ws = np.asarray(W, np.float32)[:, sl]
            return np.ascontiguousarray(
                Ws.reshape(8, P, HH).transpose(1, 0, 2)).astype(np.float16)

        wo_s = Wo[sl, :]                                              # [512, E]
        wo_r = np.ascontiguousarray(
            wo_s.reshape(4, P, E).transpose(1, 0, 2)).astype(np.float16)
        # fold the V-bias through the output projection: bo' = bo + bv @ Wo
        bo_eff = bv[sl] @ wo_s + (bo if hh == 0 else 0.0)
        bo_c = np.ascontiguousarray(bo_eff.reshape(8, P).T.astype(np.float32))
        xq_b, xk_b, xv_b = xT[b]
        maps.append({
            "xqT": xq_b,
            "xkT": xk_b,
            "xvT": xv_b,
            "wq": wcols(Wq),
            "wk": wcols(Wk),
            "wv": wcols(Wv),
            "bq_col": np.ascontiguousarray(np.asarray(bq, np.float32)[sl].reshape(4, P).T),
            "bk_col": np.ascontiguousarray(np.asarray(bk, np.float32)[sl].reshape(4, P).T),
            "wo": wo_r,
            "bo_col": bo_c,
        })
    return maps


def _assemble(results):
    outs = [results[c]["yT"] for c in range(N_CORES)]
    return np.stack([(outs[2 * b] + outs[2 * b + 1]).T for b in range(B)]).astype(np.float32)


def kernel(**inputs):
    nc = _get_nc()
    maps = _in_maps(**inputs)
    r = run_bass_kernel_spmd(nc, maps, list(range(N_CORES)))
    return _assemble(r.results)


def _ensure_ntff_hook():
    """Register the axon NTFF profiling hook (missing antenv.axon_hooks shim)."""
    import contextlib
    import ctypes
    import types

    try:
        from antenv.axon_hooks import get_axon_ntff_profile_hook
        if get_axon_ntff_profile_hook() is not None:
            return
    except ImportError:
        pass

    import antenv

    holder = {}
    mod = types.ModuleType("antenv.axon_hooks")
    mod.set_axon_ntff_profile_hook = lambda h: holder.__setitem__("h", h)
    mod.get_axon_ntff_profile_hook = lambda: holder.get("h")
    sys.modules["antenv.axon_hooks"] = mod
    antenv.axon_hooks = mod

    so_path = "/opt/axon/libaxon_pjrt.so"
    lib = ctypes.CDLL(so_path)
    if not hasattr(lib, "axon_start_nrt_profile"):
        return
    lib.axon_start_nrt_profile.argtypes = [ctypes.POINTER(ctypes.c_int64), ctypes.c_size_t]
    lib.axon_start_nrt_profile.restype = ctypes.c_int64
    lib.axon_stop_nrt_profile.argtypes = [ctypes.c_char_p]
    lib.axon_stop_nrt_profile.restype = ctypes.c_int64

    @contextlib.contextmanager
    def _hook(output_dir, device_ids):
        import jax

        jax.devices()
        if device_ids:
            ids = (ctypes.c_int64 * len(device_ids))(*device_ids)
            rc = lib.axon_start_nrt_profile(ids, len(device_ids))
        else:
            rc = lib.axon_start_nrt_profile(None, 0)
        if rc != 0:
            raise RuntimeError(f"axon_start_nrt_profile rc={rc}")
        try:
            yield
        finally:
            n = lib.axon_stop_nrt_profile(str(output_dir).encode())
            if n < 0:
                raise RuntimeError(f"axon_stop_nrt_profile rc={n}")

    mod.set_axon_ntff_profile_hook(_hook)


def kernel_traced(tmpdir=None, **inputs):
    """Like kernel() but with NTFF tracing; returns (output, exec_time_ns)."""
    _ensure_ntff_hook()
    import concourse.bass_utils as bu
    bu.upload_artifacts = lambda d: d  # no artifact bucket in this container
    nc = _get_nc()
    maps = _in_maps(**inputs)
    r = run_bass_kernel_spmd(nc, maps, list(range(N_CORES)), trace=True, tmpdir=tmpdir)
    return _assemble(r.results), r.exec_time_ns
